# revision 8
# baseline (speedup 1.0000x reference)
"""Trainium2 Bass kernel for nn_Attention_46858093199829.

Math note (why x and b are never read on-device):
    score[b,i,j] = x[b,i] @ wx + key[j] @ wk + b0
The x-dependent term and the bias are constant in j, so they cancel in
softmax over j:
    softmax_j(score[b,i,:]) = softmax(key @ wk)          (same for every b,i)
    out[b,i,:]              = softmax(key @ wk) @ value  (a single 3-vector)

Sharding: data-parallel over batch B=32 -> 4 batches per core. key/value/W
are replicated; every core computes the (identical) 3-vector and writes its
own (4,512,3) output shard.
"""

import sys
import types

import numpy as np

import concourse.bacc as bacc
import concourse.bass as bass
import concourse.tile as tile
from concourse import mybir
from concourse.bass_utils import run_bass_kernel_spmd


def _install_ntff_shim():
    """bass_utils' trace path imports antenv.axon_hooks, which some images
    lack. Provide it, backed by trn_boot's ctypes NTFF hook when available;
    returning None makes bass_utils skip tracing instead of crashing."""
    if "antenv.axon_hooks" in sys.modules:
        return
    try:
        import antenv

        m = types.ModuleType("antenv.axon_hooks")
        _state = {"hook": None, "tried": False}

        def set_axon_ntff_profile_hook(h):
            _state["hook"] = h
            _state["tried"] = True

        def get_axon_ntff_profile_hook():
            if not _state["tried"]:
                _state["tried"] = True
                try:
                    from trn_agent_boot.trn_boot import _ntff_profile_via_ctypes

                    _state["hook"] = _ntff_profile_via_ctypes(
                        "/opt/axon/libaxon_pjrt.so"
                    )
                except Exception:
                    _state["hook"] = None
            return _state["hook"]

        m.set_axon_ntff_profile_hook = set_axon_ntff_profile_hook
        m.get_axon_ntff_profile_hook = get_axon_ntff_profile_hook
        sys.modules["antenv.axon_hooks"] = m
        antenv.axon_hooks = m
    except Exception:
        pass


_install_ntff_shim()

N_CORES = 8
B, S1, S2, D = 32, 512, 2048, 3
P, NF = 128, 16  # 2048 = 128 partitions x 16 free
ROWS_PER_CORE = (B // N_CORES) * S1  # 2048 rows of (3,) per core

# Exposed for the test harness: the BassKernelResults of the last run
# (carries exec_time_ns when BASS_TRACE=1).
last_result = None

_nc_cache = None


def _build():
    # Bacc (not raw Bass): its compile() pass legalizes multi-sem waits into
    # event-semaphore instructions; raw Bass BIR fails walrus codegen with
    # "Too many sync wait commands" on any instruction waiting on >1 sem.
    nc = bacc.Bacc(target_bir_lowering=False, debug=False, num_devices=N_CORES)
    f32 = mybir.dt.float32
    key_t = nc.dram_tensor("key", [S2, D], f32, kind="ExternalInput")
    val_t = nc.dram_tensor("value", [S2, D], f32, kind="ExternalInput")
    w_t = nc.dram_tensor("W", [1, 6], f32, kind="ExternalInput")
    out_t = nc.dram_tensor("out", [ROWS_PER_CORE, D], f32, kind="ExternalOutput")

    with tile.TileContext(nc) as tc:
        with (
            tc.tile_pool(name="sb", bufs=1) as pool,
            tc.tile_pool(name="ps", bufs=1, space="PSUM") as psum,
        ):
            kv = pool.tile([P, NF, D], f32)
            vv = pool.tile([P, NF, D], f32)
            wb = pool.tile([P, 6], f32)
            nc.gpsimd.dma_start(out=kv, in_=key_t[:, :].rearrange("(p n) d -> p n d", p=P))
            nc.gpsimd.dma_start(out=vv, in_=val_t[:, :].rearrange("(p n) d -> p n d", p=P))
            nc.gpsimd.dma_start(out=wb, in_=w_t[:, :].to_broadcast([P, 6]))

            # Touch vv once so later (wait-slot-limited) consumers only
            # depend on same/single-engine producers.
            scratch = pool.tile([P, D], f32)
            nc.vector.tensor_copy(scratch, vv[:, 0, :])

            # wkrep[p, n, :] = wk = W[0, 3:6], replicated along the free dim
            wkrep = pool.tile([P, NF, D], f32)
            nc.vector.tensor_copy(wkrep[:, 0, :], wb[:, 3:6])
            for w in (1, 2, 4, 8):
                nc.vector.tensor_copy(wkrep[:, w : 2 * w, :], wkrep[:, 0:w, :])

            # sk[p,n] = sum_d key[p,n,d] * wk[d]
            prod = pool.tile([P, NF, D], f32)
            nc.vector.tensor_mul(prod, kv, wkrep)
            sk = pool.tile([P, NF], f32)
            nc.vector.reduce_sum(sk, prod, axis=mybir.AxisListType.X)

            # e = exp(sk); esum = per-partition sum of e
            e = pool.tile([P, NF], f32)
            esum = pool.tile([P, 1], f32)
            nc.scalar.activation(
                e, sk, mybir.ActivationFunctionType.Exp, accum_out=esum
            )

            # pcat[:,d] = per-partition sum of e * value[:,:,d]; pcat[:,3] = esum
            # (plain mul+reduce: InstTensorTensorReduce crashes HW on this path)
            pcat = pool.tile([P, 4], f32)
            junk = pool.tile([P, NF], f32)
            for d in range(D):
                nc.vector.tensor_mul(junk, e, vv[:, :, d])
                nc.vector.reduce_sum(
                    pcat[:, d : d + 1], junk, axis=mybir.AxisListType.X
                )
            nc.vector.tensor_copy(pcat[:, 3:4], esum)

            # Reduce across partitions: ones(128,1).T @ pcat -> (1,4)
            ones_col = pool.tile([P, 1], f32)
            nc.vector.memset(ones_col, 1.0)
            red = psum.tile([1, 4], f32)
            nc.tensor.matmul(red, ones_col, pcat, start=True, stop=True)
            red_sb = pool.tile([1, 4], f32)
            nc.vector.tensor_copy(red_sb, red)

            # Broadcast (1,4) back to all 128 partitions: ones(1,128).T @ red_sb
            ones_row = pool.tile([1, P], f32)
            nc.vector.memset(ones_row, 1.0)
            redb = psum.tile([P, 4], f32)
            nc.tensor.matmul(redb, ones_row, red_sb, start=True, stop=True)
            sb4 = pool.tile([P, 4], f32)
            nc.vector.tensor_copy(sb4, redb)

            # v* = num / Z, written into the first 3-slot of the out tile,
            # then doubled out to all 16 slots.
            rz = pool.tile([P, 1], f32)
            nc.vector.reciprocal(rz, sb4[:, 3:4])
            ot = pool.tile([P, NF, D], f32)
            nc.vector.tensor_scalar_mul(ot[:, 0, :], sb4[:, 0:3], rz)
            for w in (1, 2, 4, 8):
                nc.vector.tensor_copy(ot[:, w : 2 * w, :], ot[:, 0:w, :])

            nc.gpsimd.dma_start(
                out=out_t[:, :].rearrange("(p n) d -> p n d", p=P), in_=ot
            )
    nc.compile()
    return nc


def kernel(x, key, value, W, b):
    global last_result, _nc_cache
    key = np.ascontiguousarray(np.asarray(key, dtype=np.float32))
    value = np.ascontiguousarray(np.asarray(value, dtype=np.float32))
    W = np.ascontiguousarray(np.asarray(W, dtype=np.float32))
    if _nc_cache is None:
        _nc_cache = _build()
    in_maps = [
        {"key": key, "value": value, "W": W} for _ in range(N_CORES)
    ]
    res = run_bass_kernel_spmd(_nc_cache, in_maps, core_ids=list(range(N_CORES)))
    last_result = res
    out = np.concatenate([r["out"] for r in res.results], axis=0)
    return out.reshape(B, S1, D)
